# revision 39
# baseline (speedup 1.0000x reference)
"""Trainium2 Bass kernel for nn_CompactControlAttention.

The module's attention is degenerate: softmax over a size-1 axis is exactly
1.0, so queries/keys (Wq, bq, Wk, bk) never affect the output:

    out[b, s, :] = sequence[b, s, :] + p[b, :]
    p = (sum_c controls[c]) @ Wv.T @ Wo.T + C * (bv @ Wo.T + bo)

Sharding: tensor-parallel over the hidden feature dim f of v = cs @ Wv.T
and over the output feature dim e of p. Cross-core exchange of the tiny
v.T (256KB) happens between two NEFF launches via host gather -- on-chip
collectives cost ~75us of fixed setup per execute on this stack, and HBM
is only pair-shared, so a host hop is the cheapest 8-way exchange.

NEFF-1 (per core k, ~3MB DMA):
  cs_t = sum_c controls_t[c]      (controls shipped pre-transposed, bf16)
  v_k  = cs @ Wv.T[:, fk] + C*bv  (16 bf16 matmuls, 256-wide, PSUM accum)
  vt_k = v_k.T                    (2 PE transposes) -> out [256, 64] bf16

NEFF-2 (per core k, ~4.3MB DMA; host feeds the gathered full v.T):
  p_k  = v @ Wo.T[:, ek] + bo     (16 bf16 matmuls)
  out  = seq_k + broadcast_s(p_k) (chunked DVE/GpSimd adds, piped DMA)
"""

import numpy as np
import ml_dtypes

import concourse.bass as bass
import concourse.mybir as mybir
import concourse.tile as tile
from concourse import bacc
from concourse.bass_utils import run_bass_kernel_spmd
from concourse.masks import make_identity

N_CORES = 8
D = 2048
B = 64
S = 32
C = 8
EK = D // N_CORES  # 256
NT = D // 128  # 16
F32 = mybir.dt.float32
BF16 = mybir.dt.bfloat16

_CACHE = {}


# --------------------------- NEFF-1: v.T slice ---------------------------


def _build_nc1():
    # All big inputs ship partition-major [128, N] so every DMA line is a
    # long contiguous DRAM run (8-16KB) instead of 512B-1KB scatter reads.
    nc = bacc.Bacc("TRN2", target_bir_lowering=False, debug=False, num_devices=N_CORES)
    ctrl = nc.dram_tensor("ctrl", [128, NT * C * B], BF16, kind="ExternalInput")
    wvt = nc.dram_tensor("wvt", [128, NT * EK], BF16, kind="ExternalInput")
    bv = nc.dram_tensor("bv", [EK], F32, kind="ExternalInput")
    vt_out = nc.dram_tensor("vt", [128, 2 * B], BF16, kind="ExternalOutput")

    with tile.TileContext(nc) as tc:
        from contextlib import ExitStack

        ctx = ExitStack()
        P = 128
        consts = ctx.enter_context(tc.tile_pool(name="consts", bufs=1))
        sbuf = ctx.enter_context(tc.tile_pool(name="sbuf", bufs=1))
        psum_v = ctx.enter_context(tc.tile_pool(name="psum_v", bufs=1, space="PSUM"))
        psum_t = ctx.enter_context(tc.tile_pool(name="psum_t", bufs=1, space="PSUM"))

        # ctrl eighths interleaved with wv quarters on both HWDGE queues:
        # MM1 tile t needs cs eighth t//2 and wv quarter t//4, so pairing
        # them lets the PE chase the DMA stream.
        ctrl_sb = sbuf.tile([P, NT * C * B], BF16)
        c3 = ctrl_sb[:].rearrange("p (t cb) -> p t cb", cb=C * B)
        wv_sb = sbuf.tile([P, NT * EK], BF16)
        wv4 = wv_sb[:].rearrange("p (q t f) -> p q (t f)", q=4, f=EK)
        Q = NT // 8  # 2 t-tiles per ctrl piece

        CW = Q * C * B  # 1024: ctrl columns per eighth
        WW = 4 * EK  # 1024: wv columns per quarter

        def ctrl_piece(qi, q):
            q.dma_start(
                out=ctrl_sb[:, qi * CW : (qi + 1) * CW],
                in_=ctrl[:, qi * CW : (qi + 1) * CW],
            )

        def wv_piece(qi, q):
            q.dma_start(
                out=wv_sb[:, qi * WW : (qi + 1) * WW],
                in_=wvt[:, qi * WW : (qi + 1) * WW],
            )

        for qi in range(8):
            ctrl_piece(qi, nc.sync if qi % 2 == 0 else nc.scalar)
        for qi in range(4):
            wv_piece(qi, nc.sync if qi % 2 == 0 else nc.scalar)
        bv_sb = consts.tile([1, EK], F32)
        nc.gpsimd.dma_start(out=bv_sb[:], in_=bv[None, :])

        ident = consts.tile([P, P], F32)
        make_identity(nc, ident[:])
        ident_b = consts.tile([P, P], BF16)
        nc.vector.tensor_copy(ident_b[:], ident[:])
        ones8_f = consts.tile([1, B], F32)
        nc.vector.memset(ones8_f[:], float(C))
        ones8 = consts.tile([1, B], BF16)
        nc.vector.tensor_copy(ones8[:], ones8_f[:])
        bv_b = consts.tile([1, EK], BF16)
        nc.vector.tensor_copy(bv_b[:], bv_sb[:])

        # cs tree sum, per ctrl eighth as it lands (DVE + GpSimd split)
        c4 = ctrl_sb[:].rearrange("p (t c b) -> p t c b", c=C, b=B)
        s1 = sbuf.tile([P, NT * 4 * B], BF16)
        s1v = s1[:].rearrange("p (t c b) -> p t c b", c=4, b=B)
        s2 = sbuf.tile([P, NT * 2 * B], BF16)
        s2v = s2[:].rearrange("p (t c b) -> p t c b", c=2, b=B)
        cs = sbuf.tile([P, NT * B], BF16)
        csv = cs[:].rearrange("p (t b) -> p t b", b=B)
        for qi in range(8):
            ts = slice(qi * Q, (qi + 1) * Q)
            # GpSimd is ~2x slower at tensor_add than DVE: give it 2/8
            eng = nc.gpsimd if qi in (1, 5) else nc.vector
            eng.tensor_add(s1v[:, ts], c4[:, ts, 0:4, :], c4[:, ts, 4:8, :])
            eng.tensor_add(s2v[:, ts], s1v[:, ts, 0:2, :], s1v[:, ts, 2:4, :])
            eng.tensor_add(csv[:, ts], s2v[:, ts, 0, :], s2v[:, ts, 1, :])

        # PE warmup: the clock ramps 1.2->2.4GHz after ~3us of continuous
        # busy; burn the DMA-wait window on dummy matmuls so MM1 runs hot.
        wp = psum_t.tile([P, P], F32, tag="wp", name="wp")
        for _ in range(28):
            nc.tensor.matmul(wp[:], ident_b[:], ident_b[:], start=True, stop=True)

        # MM1 + bias; GpSimd-summed eighths (t 2,3,10,11) accumulate last
        # so their slower cs tiles never stall the PE queue.
        pv = psum_v.tile([B, EK], F32, tag="pv")
        wv3 = wv_sb[:].rearrange("p (t f) -> p t f", f=EK)
        order = [t for t in range(NT) if t not in (2, 3, 10, 11)] + [2, 3, 10, 11]
        for i, t in enumerate(order):
            nc.tensor.matmul(
                pv[:], csv[:, t, :], wv3[:, t, :], start=(i == 0), stop=False
            )
        nc.tensor.matmul(pv[:], ones8[:], bv_b[:], start=False, stop=True)
        v = sbuf.tile([B, EK], BF16)
        nc.vector.tensor_copy(v[:], pv[:])

        # vt = v.T
        pt = psum_t.tile([P, 2 * B], BF16, tag="pt")
        for g in range(2):
            nc.tensor.transpose(
                pt[:, g * B : (g + 1) * B], v[:, g * 128 : (g + 1) * 128],
                ident_b[0:B, 0:B],
            )
        vt = sbuf.tile([P, 2 * B], BF16)
        nc.vector.tensor_copy(vt[:], pt[:])
        nc.sync.dma_start(out=vt_out[:], in_=vt[:])
        ctx.close()
    nc.compile()
    return nc


# ------------------------ NEFF-2: MM2 + residual -------------------------


def _build_nc2():
    nc = bacc.Bacc("TRN2", target_bir_lowering=False, debug=False, num_devices=N_CORES)
    vta = nc.dram_tensor("vta", [128, NT * B], BF16, kind="ExternalInput")
    wot = nc.dram_tensor("wot", [128, NT * EK], BF16, kind="ExternalInput")
    bo = nc.dram_tensor("bo", [EK], F32, kind="ExternalInput")
    seq = nc.dram_tensor("seq", [128, S * 128], BF16, kind="ExternalInput")
    out = nc.dram_tensor("out", [128, S * 128], F32, kind="ExternalOutput")

    with tile.TileContext(nc) as tc:
        from contextlib import ExitStack

        ctx = ExitStack()
        P = 128
        consts = ctx.enter_context(tc.tile_pool(name="consts", bufs=1))
        sbuf = ctx.enter_context(tc.tile_pool(name="sbuf", bufs=1))
        psum_p = ctx.enter_context(tc.tile_pool(name="psum_p", bufs=1, space="PSUM"))

        vta_sb = sbuf.tile([P, NT * B], BF16)
        vta3 = vta_sb[:].rearrange("p (t b) -> p t b", b=B)
        nc.sync.dma_start(out=vta_sb[:], in_=vta[:])
        # wot first on BOTH queues (halves) -- MM2's critical input
        wot_sb = sbuf.tile([P, NT * EK], BF16)
        HW = 8 * EK  # 2048: wot columns per half
        for hi in range(2):
            q = nc.scalar if hi == 0 else nc.sync
            q.dma_start(
                out=wot_sb[:, hi * HW : (hi + 1) * HW],
                in_=wot[:, hi * HW : (hi + 1) * HW],
            )
        bo_sb = consts.tile([1, EK], F32)
        nc.gpsimd.dma_start(out=bo_sb[:], in_=bo[None, :])
        # seq entirely on sync (behind wot-h1) so scalar carries ONLY
        # wot-h0 -- MM2's first-needed input lands ~3us earlier.
        seq_sb = sbuf.tile([P, S * 128], BF16)
        nc.sync.dma_start(out=seq_sb[:, 0 : S * 64], in_=seq[:, 0 : S * 64])
        nc.sync.dma_start(out=seq_sb[:, S * 64 :], in_=seq[:, S * 64 :])

        ones1_f = consts.tile([1, B], F32)
        nc.vector.memset(ones1_f[:], 1.0)
        ones1 = consts.tile([1, B], BF16)
        nc.vector.tensor_copy(ones1[:], ones1_f[:])
        bo_b = consts.tile([1, EK], BF16)
        nc.vector.tensor_copy(bo_b[:], bo_sb[:])
        identw = consts.tile([P, P], BF16)
        nc.vector.memset(identw[:], 0.001)

        # PE warmup while wot streams (clock ramp, see NEFF-1)
        wp = psum_p.tile([P, P], F32, tag="wp", name="wp")
        for _ in range(16):
            nc.tensor.matmul(wp[:], identw[:], identw[:], start=True, stop=True)

        pp = psum_p.tile([B, EK], F32, tag="pp")
        wo3 = wot_sb[:].rearrange("p (t e) -> p t e", e=EK)
        for t in range(NT):
            nc.tensor.matmul(
                pp[:], vta3[:, t, :], wo3[:, t, :], start=(t == 0), stop=False
            )
        nc.tensor.matmul(pp[:], ones1[:], bo_b[:], start=False, stop=True)

        p_re = sbuf.tile([P, P], F32)
        nc.vector.tensor_copy(p_re[0:B, :], pp[:, 0:P])
        nc.vector.tensor_copy(p_re[B : 2 * B, :], pp[:, P : 2 * P])

        out_sb = sbuf.tile([P, S * 128], F32)
        o3 = out_sb[:].rearrange("p (s e) -> p s e", e=P)
        q3 = seq_sb[:].rearrange("p (s e) -> p s e", e=P)
        chunks = [  # (engine, s0, s1, queue)
            (nc.gpsimd, 28, 32, nc.scalar),
            (nc.vector, 0, 10, nc.sync),
            (nc.vector, 10, 20, nc.scalar),
            (nc.vector, 20, 28, nc.sync),
        ]
        for eng, s0, s1, q in chunks:
            eng.tensor_add(
                o3[:, s0:s1, :], q3[:, s0:s1, :],
                p_re[:, None, :].to_broadcast((P, s1 - s0, P)),
            )
            q.dma_start(
                out=out[:, s0 * 128 : s1 * 128], in_=out_sb[:, s0 * 128 : s1 * 128]
            )
        ctx.close()
    nc.compile()
    return nc


def _get_ncs():
    if "nc1" not in _CACHE:
        _CACHE["nc1"] = _build_nc1()
        _CACHE["nc2"] = _build_nc2()
    return _CACHE["nc1"], _CACHE["nc2"]


def _pmaj(a, w):
    """(nt*128, w) row-major -> (128, nt*w) partition-major contiguous."""
    nt = a.shape[0] // 128
    return np.ascontiguousarray(
        a.reshape(nt, 128, w).transpose(1, 0, 2).reshape(128, nt * w)
    )


def _run(inputs, trace=False):
    nc1, nc2 = _get_ncs()
    bf = ml_dtypes.bfloat16
    sequence = np.asarray(inputs["sequence"])
    controls = np.asarray(inputs["controls"])
    Wv = np.asarray(inputs["Wv"])
    bv = np.asarray(inputs["bv"])
    Wo = np.asarray(inputs["Wo"])
    bo = np.asarray(inputs["bo"])

    ctrl_t = _pmaj(controls.transpose(2, 0, 1).reshape(D, C * B).astype(bf), C * B)
    in1 = []
    for k in range(N_CORES):
        fk = slice(k * EK, (k + 1) * EK)
        in1.append(
            {
                "ctrl": ctrl_t,
                "wvt": _pmaj(Wv[fk, :].T.astype(bf), EK),
                "bv": np.ascontiguousarray(bv[fk]),
            }
        )
    res1 = run_bass_kernel_spmd(nc1, in1, list(range(N_CORES)), trace=trace)

    # assemble v.T partition-major: column block t = 2k+g is core k's g-half
    vta = np.empty((128, NT * B), dtype=bf)
    for k in range(N_CORES):
        r = np.asarray(res1.results[k]["vt"])  # (128, 2*B): [p, (g b)]
        vta[:, (2 * k) * B : (2 * k + 1) * B] = r[:, 0:B]
        vta[:, (2 * k + 1) * B : (2 * k + 2) * B] = r[:, B : 2 * B]

    in2 = []
    for k in range(N_CORES):
        ek = slice(k * EK, (k + 1) * EK)
        in2.append(
            {
                "vta": vta,
                "wot": _pmaj(Wo[ek, :].T.astype(bf), EK),
                "bo": np.ascontiguousarray(bo[ek]),
                "seq": np.ascontiguousarray(
                    sequence[:, :, ek]
                    .reshape(B, S, 2, 128)
                    .transpose(2, 0, 1, 3)
                    .reshape(128, S * 128)
                    .astype(bf)
                ),
            }
        )
    res2 = run_bass_kernel_spmd(nc2, in2, list(range(N_CORES)), trace=trace)

    out = np.empty((B, S, D), dtype=np.float32)
    for k in range(N_CORES):
        out[:, :, k * EK : (k + 1) * EK] = (
            res2.results[k]["out"]
            .reshape(2, B, S, 128)
            .transpose(1, 2, 0, 3)
            .reshape(B, S, EK)
        )
    return out, (res1, res2)


def kernel(**inputs):
    out, _ = _run(inputs)
    return out


# revision 41
# speedup vs baseline: 1.0446x; 1.0446x over previous
"""Trainium2 Bass kernel for nn_CompactControlAttention.

The module's attention is degenerate: softmax over a size-1 axis is exactly
1.0, so queries/keys (Wq, bq, Wk, bk) never affect the output:

    out[b, s, :] = sequence[b, s, :] + p[b, :]
    p = (sum_c controls[c]) @ Wv.T @ Wo.T + C * (bv @ Wo.T + bo)

Sharding: tensor-parallel over the hidden feature dim f of v = cs @ Wv.T
and over the output feature dim e of p. Cross-core exchange of the tiny
v.T (256KB) happens between two NEFF launches via host gather -- on-chip
collectives cost ~75us of fixed setup per execute on this stack, and HBM
is only pair-shared, so a host hop is the cheapest 8-way exchange.

NEFF-1 (per core k, ~3MB DMA):
  cs_t = sum_c controls_t[c]      (controls shipped pre-transposed, bf16)
  v_k  = cs @ Wv.T[:, fk] + C*bv  (16 bf16 matmuls, 256-wide, PSUM accum)
  vt_k = v_k.T                    (2 PE transposes) -> out [256, 64] bf16

NEFF-2 (per core k, ~4.3MB DMA; host feeds the gathered full v.T):
  p_k  = v @ Wo.T[:, ek] + bo     (16 bf16 matmuls)
  out  = seq_k + broadcast_s(p_k) (chunked DVE/GpSimd adds, piped DMA)
"""

import numpy as np
import ml_dtypes

import concourse.bass as bass
import concourse.mybir as mybir
import concourse.tile as tile
from concourse import bacc
from concourse.bass_utils import run_bass_kernel_spmd
from concourse.masks import make_identity

N_CORES = 8
D = 2048
B = 64
S = 32
C = 8
EK = D // N_CORES  # 256
NT = D // 128  # 16
F32 = mybir.dt.float32
BF16 = mybir.dt.bfloat16

_CACHE = {}


# --------------------------- NEFF-1: v.T slice ---------------------------


def _build_nc1():
    # All big inputs ship partition-major [128, N] so every DMA line is a
    # long contiguous DRAM run (8-16KB) instead of 512B-1KB scatter reads.
    nc = bacc.Bacc("TRN2", target_bir_lowering=False, debug=False, num_devices=N_CORES)
    ctrl = nc.dram_tensor("ctrl", [128, NT * C * B], BF16, kind="ExternalInput")
    wvt = nc.dram_tensor("wvt", [128, NT * EK], BF16, kind="ExternalInput")
    bv = nc.dram_tensor("bv", [EK], F32, kind="ExternalInput")
    vt_out = nc.dram_tensor("vt", [128, 2 * B], BF16, kind="ExternalOutput")

    with tile.TileContext(nc) as tc:
        from contextlib import ExitStack

        ctx = ExitStack()
        P = 128
        consts = ctx.enter_context(tc.tile_pool(name="consts", bufs=1))
        sbuf = ctx.enter_context(tc.tile_pool(name="sbuf", bufs=1))
        psum_v = ctx.enter_context(tc.tile_pool(name="psum_v", bufs=1, space="PSUM"))
        psum_t = ctx.enter_context(tc.tile_pool(name="psum_t", bufs=1, space="PSUM"))

        # ctrl eighths interleaved with wv quarters on both HWDGE queues:
        # MM1 tile t needs cs eighth t//2 and wv quarter t//4, so pairing
        # them lets the PE chase the DMA stream.
        ctrl_sb = sbuf.tile([P, NT * C * B], BF16)
        c3 = ctrl_sb[:].rearrange("p (t cb) -> p t cb", cb=C * B)
        wv_sb = sbuf.tile([P, NT * EK], BF16)
        wv4 = wv_sb[:].rearrange("p (q t f) -> p q (t f)", q=4, f=EK)
        Q = NT // 8  # 2 t-tiles per ctrl piece

        CW = Q * C * B  # 1024: ctrl columns per eighth
        WW = 4 * EK  # 1024: wv columns per quarter

        def ctrl_piece(qi, q):
            q.dma_start(
                out=ctrl_sb[:, qi * CW : (qi + 1) * CW],
                in_=ctrl[:, qi * CW : (qi + 1) * CW],
            )

        def wv_piece(qi, q):
            q.dma_start(
                out=wv_sb[:, qi * WW : (qi + 1) * WW],
                in_=wvt[:, qi * WW : (qi + 1) * WW],
            )

        for qi in range(8):
            ctrl_piece(qi, nc.sync if qi % 2 == 0 else nc.scalar)
        for qi in range(4):
            wv_piece(qi, nc.sync if qi % 2 == 0 else nc.scalar)
        bv_sb = consts.tile([1, EK], F32)
        nc.gpsimd.dma_start(out=bv_sb[:], in_=bv[None, :])

        ident = consts.tile([P, P], F32)
        make_identity(nc, ident[:])
        ident_b = consts.tile([P, P], BF16)
        nc.vector.tensor_copy(ident_b[:], ident[:])
        ones8_f = consts.tile([1, B], F32)
        nc.vector.memset(ones8_f[:], float(C))
        ones8 = consts.tile([1, B], BF16)
        nc.vector.tensor_copy(ones8[:], ones8_f[:])
        bv_b = consts.tile([1, EK], BF16)
        nc.vector.tensor_copy(bv_b[:], bv_sb[:])

        # cs tree sum, per ctrl eighth as it lands (DVE + GpSimd split)
        c4 = ctrl_sb[:].rearrange("p (t c b) -> p t c b", c=C, b=B)
        s1 = sbuf.tile([P, NT * 4 * B], BF16)
        s1v = s1[:].rearrange("p (t c b) -> p t c b", c=4, b=B)
        s2 = sbuf.tile([P, NT * 2 * B], BF16)
        s2v = s2[:].rearrange("p (t c b) -> p t c b", c=2, b=B)
        cs = sbuf.tile([P, NT * B], BF16)
        csv = cs[:].rearrange("p (t b) -> p t b", b=B)
        for qi in range(8):
            ts = slice(qi * Q, (qi + 1) * Q)
            # GpSimd is ~2x slower at tensor_add than DVE: give it 2/8
            eng = nc.gpsimd if qi in (1, 5) else nc.vector
            eng.tensor_add(s1v[:, ts], c4[:, ts, 0:4, :], c4[:, ts, 4:8, :])
            eng.tensor_add(s2v[:, ts], s1v[:, ts, 0:2, :], s1v[:, ts, 2:4, :])
            eng.tensor_add(csv[:, ts], s2v[:, ts, 0, :], s2v[:, ts, 1, :])

        # MM1 + bias; GpSimd-summed eighths (t 2,3,10,11) accumulate last
        # so their slower cs tiles never stall the PE queue.
        pv = psum_v.tile([B, EK], F32, tag="pv")
        wv3 = wv_sb[:].rearrange("p (t f) -> p t f", f=EK)
        order = [t for t in range(NT) if t not in (2, 3, 10, 11)] + [2, 3, 10, 11]
        for i, t in enumerate(order):
            nc.tensor.matmul(
                pv[:], csv[:, t, :], wv3[:, t, :], start=(i == 0), stop=False
            )
        nc.tensor.matmul(pv[:], ones8[:], bv_b[:], start=False, stop=True)
        v = sbuf.tile([B, EK], BF16)
        nc.vector.tensor_copy(v[:], pv[:])

        # vt = v.T
        pt = psum_t.tile([P, 2 * B], BF16, tag="pt")
        for g in range(2):
            nc.tensor.transpose(
                pt[:, g * B : (g + 1) * B], v[:, g * 128 : (g + 1) * 128],
                ident_b[0:B, 0:B],
            )
        vt = sbuf.tile([P, 2 * B], BF16)
        nc.vector.tensor_copy(vt[:], pt[:])
        nc.sync.dma_start(out=vt_out[:], in_=vt[:])
        ctx.close()
    nc.compile()
    return nc


# ------------------------ NEFF-2: MM2 + residual -------------------------


def _build_nc2():
    nc = bacc.Bacc("TRN2", target_bir_lowering=False, debug=False, num_devices=N_CORES)
    vta = nc.dram_tensor("vta", [128, NT * B], BF16, kind="ExternalInput")
    wot = nc.dram_tensor("wot", [128, NT * EK], BF16, kind="ExternalInput")
    bo = nc.dram_tensor("bo", [EK], F32, kind="ExternalInput")
    seq = nc.dram_tensor("seq", [128, S * 128], BF16, kind="ExternalInput")
    out = nc.dram_tensor("out", [128, S * 128], F32, kind="ExternalOutput")

    with tile.TileContext(nc) as tc:
        from contextlib import ExitStack

        ctx = ExitStack()
        P = 128
        consts = ctx.enter_context(tc.tile_pool(name="consts", bufs=1))
        sbuf = ctx.enter_context(tc.tile_pool(name="sbuf", bufs=1))
        psum_p = ctx.enter_context(tc.tile_pool(name="psum_p", bufs=1, space="PSUM"))

        vta_sb = sbuf.tile([P, NT * B], BF16)
        vta3 = vta_sb[:].rearrange("p (t b) -> p t b", b=B)
        nc.sync.dma_start(out=vta_sb[:], in_=vta[:])
        # wot first on BOTH queues (halves) -- MM2's critical input
        wot_sb = sbuf.tile([P, NT * EK], BF16)
        HW = 8 * EK  # 2048: wot columns per half
        for hi in range(2):
            q = nc.scalar if hi == 0 else nc.sync
            q.dma_start(
                out=wot_sb[:, hi * HW : (hi + 1) * HW],
                in_=wot[:, hi * HW : (hi + 1) * HW],
            )
        bo_sb = consts.tile([1, EK], F32)
        nc.gpsimd.dma_start(out=bo_sb[:], in_=bo[None, :])
        # seq entirely on sync (behind wot-h1) so scalar carries ONLY
        # wot-h0 -- MM2's first-needed input lands ~3us earlier.
        seq_sb = sbuf.tile([P, S * 128], BF16)
        nc.sync.dma_start(out=seq_sb[:, 0 : S * 64], in_=seq[:, 0 : S * 64])
        nc.sync.dma_start(out=seq_sb[:, S * 64 :], in_=seq[:, S * 64 :])

        ones1_f = consts.tile([1, B], F32)
        nc.vector.memset(ones1_f[:], 1.0)
        ones1 = consts.tile([1, B], BF16)
        nc.vector.tensor_copy(ones1[:], ones1_f[:])
        bo_b = consts.tile([1, EK], BF16)
        nc.vector.tensor_copy(bo_b[:], bo_sb[:])
        pp = psum_p.tile([B, EK], F32, tag="pp")
        wo3 = wot_sb[:].rearrange("p (t e) -> p t e", e=EK)
        for t in range(NT):
            nc.tensor.matmul(
                pp[:], vta3[:, t, :], wo3[:, t, :], start=(t == 0), stop=False
            )
        nc.tensor.matmul(pp[:], ones1[:], bo_b[:], start=False, stop=True)

        p_re = sbuf.tile([P, P], F32)
        nc.vector.tensor_copy(p_re[0:B, :], pp[:, 0:P])
        nc.vector.tensor_copy(p_re[B : 2 * B, :], pp[:, P : 2 * P])

        out_sb = sbuf.tile([P, S * 128], F32)
        o3 = out_sb[:].rearrange("p (s e) -> p s e", e=P)
        q3 = seq_sb[:].rearrange("p (s e) -> p s e", e=P)
        chunks = [  # (engine, s0, s1, queue)
            (nc.gpsimd, 28, 32, nc.scalar),
            (nc.vector, 0, 10, nc.sync),
            (nc.vector, 10, 20, nc.scalar),
            (nc.vector, 20, 28, nc.sync),
        ]
        for eng, s0, s1, q in chunks:
            eng.tensor_add(
                o3[:, s0:s1, :], q3[:, s0:s1, :],
                p_re[:, None, :].to_broadcast((P, s1 - s0, P)),
            )
            q.dma_start(
                out=out[:, s0 * 128 : s1 * 128], in_=out_sb[:, s0 * 128 : s1 * 128]
            )
        ctx.close()
    nc.compile()
    return nc


def _get_ncs():
    if "nc1" not in _CACHE:
        _CACHE["nc1"] = _build_nc1()
        _CACHE["nc2"] = _build_nc2()
    return _CACHE["nc1"], _CACHE["nc2"]


def _pmaj(a, w):
    """(nt*128, w) row-major -> (128, nt*w) partition-major contiguous."""
    nt = a.shape[0] // 128
    return np.ascontiguousarray(
        a.reshape(nt, 128, w).transpose(1, 0, 2).reshape(128, nt * w)
    )


def _run(inputs, trace=False):
    nc1, nc2 = _get_ncs()
    bf = ml_dtypes.bfloat16
    sequence = np.asarray(inputs["sequence"])
    controls = np.asarray(inputs["controls"])
    Wv = np.asarray(inputs["Wv"])
    bv = np.asarray(inputs["bv"])
    Wo = np.asarray(inputs["Wo"])
    bo = np.asarray(inputs["bo"])

    ctrl_t = _pmaj(controls.transpose(2, 0, 1).reshape(D, C * B).astype(bf), C * B)
    in1 = []
    for k in range(N_CORES):
        fk = slice(k * EK, (k + 1) * EK)
        in1.append(
            {
                "ctrl": ctrl_t,
                "wvt": _pmaj(Wv[fk, :].T.astype(bf), EK),
                "bv": np.ascontiguousarray(bv[fk]),
            }
        )
    res1 = run_bass_kernel_spmd(nc1, in1, list(range(N_CORES)), trace=trace)

    # assemble v.T partition-major: column block t = 2k+g is core k's g-half
    vta = np.empty((128, NT * B), dtype=bf)
    for k in range(N_CORES):
        r = np.asarray(res1.results[k]["vt"])  # (128, 2*B): [p, (g b)]
        vta[:, (2 * k) * B : (2 * k + 1) * B] = r[:, 0:B]
        vta[:, (2 * k + 1) * B : (2 * k + 2) * B] = r[:, B : 2 * B]

    in2 = []
    for k in range(N_CORES):
        ek = slice(k * EK, (k + 1) * EK)
        in2.append(
            {
                "vta": vta,
                "wot": _pmaj(Wo[ek, :].T.astype(bf), EK),
                "bo": np.ascontiguousarray(bo[ek]),
                "seq": np.ascontiguousarray(
                    sequence[:, :, ek]
                    .reshape(B, S, 2, 128)
                    .transpose(2, 0, 1, 3)
                    .reshape(128, S * 128)
                    .astype(bf)
                ),
            }
        )
    res2 = run_bass_kernel_spmd(nc2, in2, list(range(N_CORES)), trace=trace)

    out = np.empty((B, S, D), dtype=np.float32)
    for k in range(N_CORES):
        out[:, :, k * EK : (k + 1) * EK] = (
            res2.results[k]["out"]
            .reshape(2, B, S, 128)
            .transpose(1, 2, 0, 3)
            .reshape(B, S, EK)
        )
    return out, (res1, res2)


def kernel(**inputs):
    out, _ = _run(inputs)
    return out
